# revision 2
# baseline (speedup 1.0000x reference)
"""Trainium2 Bass kernel for nn_BiVision_VQA2 (B=64,T=32,D=768,N=901).

Data-parallel over batch: 8 batch elems per core x 8 cores.
Key math simplifications (validated vs reference in numpy, rel err ~1e-6):
  - ga/go attention use a single key token -> softmax==1 -> those paths are
    linear in cls; question_embeds is mathematically unused.
  - GRU input `a` is constant over time; wx computed once.
  - local attention: scores = (qemb @ W0_h) @ W1_h^T / sqrt(dk) @ X^T ;
    row-constant score terms (K bias, Q.b1) drop out of softmax; query
    pooling applied to the attention matrix before the @X contraction;
    constant bias terms folded into one vector.
"""

import numpy as np
from contextlib import ExitStack

import concourse.bass as bass
import concourse.tile as tile
from concourse import bacc, mybir
from concourse.bass_utils import run_bass_kernel_spmd
from concourse.masks import make_identity

FP = mybir.dt.float32
FPR = mybir.dt.float32r
OP = mybir.AluOpType
AF = mybir.ActivationFunctionType
BF = mybir.dt.bfloat16

NCORES = 8
BL = 8
D = 768
T = 32
G = 3 * D
NK = 900
NH = 2
DK = 384
ET = D // 128
RQ = BL * T
USE_FPR = True


def chunks(total):
    out, o = [], 0
    while o < total:
        w = min(512, total - o)
        out.append((o, w))
        o += w
    return out


CH_G = chunks(G)
CH_NK = [(0, 512), (512, 388)]
CH_D = [(0, 512), (512, 256)]


def _r(ap):
    return ap.bitcast(FPR) if USE_FPR else ap


def kchunks(n):
    out, o = [], 0
    while o < n:
        out.append((o, min(128, n - o)))
        o += 128
    return out


import os
PHASES = int(os.environ.get("KPHASES", "4"))


def build():
    nc = bacc.Bacc("TRN2", target_bir_lowering=False, debug=False,
                   enable_asserts=False)

    img = nc.dram_tensor("img", [BL, 901, D], FP, kind="ExternalInput").ap()
    h0 = nc.dram_tensor("h0", [BL, D], FP, kind="ExternalInput").ap()
    w_ih = nc.dram_tensor("gru_w_ih", [G, D], FP, kind="ExternalInput").ap()
    w_hh = nc.dram_tensor("gru_w_hh", [G, D], FP, kind="ExternalInput").ap()
    b_ih = nc.dram_tensor("gru_b_ih", [G], FP, kind="ExternalInput").ap()
    b_hh = nc.dram_tensor("gru_b_hh", [G], FP, kind="ExternalInput").ap()
    ga_w = nc.dram_tensor("ga_w", [4, D, D], FP, kind="ExternalInput").ap()
    ga_b = nc.dram_tensor("ga_b", [4, D], FP, kind="ExternalInput").ap()
    ga_pool = nc.dram_tensor("ga_pool", [1], FP, kind="ExternalInput").ap()
    la_w = nc.dram_tensor("la_w", [4, D, D], FP, kind="ExternalInput").ap()
    la_b = nc.dram_tensor("la_b", [4, D], FP, kind="ExternalInput").ap()
    la_pool = nc.dram_tensor("la_pool", [T], FP, kind="ExternalInput").ap()
    go_w = nc.dram_tensor("go_w", [4, D, D], FP, kind="ExternalInput").ap()
    go_b = nc.dram_tensor("go_b", [4, D], FP, kind="ExternalInput").ap()
    go_pool = nc.dram_tensor("go_pool", [T], FP, kind="ExternalInput").ap()
    f1_w = nc.dram_tensor("f1_w", [2 * D, 1024], FP, kind="ExternalInput").ap()
    f1_b = nc.dram_tensor("f1_b", [1024], FP, kind="ExternalInput").ap()
    f2_w = nc.dram_tensor("f2_w", [1024, 512], FP, kind="ExternalInput").ap()
    f2_b = nc.dram_tensor("f2_b", [512], FP, kind="ExternalInput").ap()
    f3_w = nc.dram_tensor("f3_w", [512, 1024], FP, kind="ExternalInput").ap()
    f3_b = nc.dram_tensor("f3_b", [1024], FP, kind="ExternalInput").ap()
    out_d = nc.dram_tensor("out", [BL, 1024], FP, kind="ExternalOutput").ap()

    with tile.TileContext(nc) as tc, ExitStack() as ctx:
        cpool = ctx.enter_context(tc.tile_pool(name="const", bufs=1))
        gstate = ctx.enter_context(tc.tile_pool(name="gstate", bufs=2))
        persist = ctx.enter_context(tc.tile_pool(name="persist", bufs=1))
        psA = ctx.enter_context(tc.tile_pool(name="psA", bufs=1, space="PSUM"))
        psB = ctx.enter_context(tc.tile_pool(name="psB", bufs=2, space="PSUM"))
        psC = ctx.enter_context(tc.tile_pool(name="psC", bufs=1, space="PSUM"))

        ident = cpool.tile([128, 128], FP, tag="ident")
        make_identity(nc, ident[:])
        ones1 = cpool.tile([1, 128], FP, tag="ones1")
        nc.vector.memset(ones1[:], 1.0)
        onesT = cpool.tile([T, 128], FP, tag="onesT")
        nc.vector.memset(onesT[:], 1.0)
        identr = cpool.tile([128, 128], FP, tag="identr")
        nc.vector.tensor_copy(_r(identr[:]), ident[:])
        identb = cpool.tile([128, 128], BF, tag="identb")
        nc.vector.tensor_copy(identb[:], ident[:])
        ones1r = cpool.tile([1, 128], FP, tag="ones1r")
        nc.vector.tensor_copy(_r(ones1r[:]), ones1[:])

        def colvec(dram_1d, n, tag):
            nt = n // 128
            t_ = cpool.tile([128, nt], FP, tag=tag)
            for j in range(nt):
                nc.sync.dma_start(t_[:, j:j + 1], dram_1d[j * 128:(j + 1) * 128][:, None])
            return t_

        b2gaT = colvec(ga_b[2], D, "b2gaT")
        b3gaT = colvec(ga_b[3], D, "b3gaT")
        b2goT = colvec(go_b[2], D, "b2goT")
        b3goT = colvec(go_b[3], D, "b3goT")
        b0laT = colvec(la_b[0], D, "b0laT")
        b2laT = colvec(la_b[2], D, "b2laT")
        b3laT = colvec(la_b[3], D, "b3laT")
        b1fT = colvec(f1_b, 1024, "b1fT")
        b2fT = colvec(f2_b, 512, "b2fT")
        b3fT = colvec(f3_b, 1024, "b3fT")

        lapool_c = cpool.tile([T, 1], FP, tag="lapool_c")
        nc.sync.dma_start(lapool_c[:], la_pool[:][:, None])
        gopool_c = cpool.tile([T, 1], FP, tag="gopool_c")
        nc.sync.dma_start(gopool_c[:], go_pool[:][:, None])
        gapool_c = cpool.tile([1, 1], FP, tag="gapool_c")
        nc.sync.dma_start(gapool_c[:], ga_pool[:][:, None])

        def sum_bcast(vcol, k, tag):
            p = psC.tile([128, 1], FP, tag="pd")
            lhs = onesT if k == T else ones1
            nc.tensor.matmul(p[:], lhs[:k, :], vcol[:k, :], start=True, stop=True)
            s = cpool.tile([128, 1], FP, tag=tag)
            nc.vector.tensor_copy(s[:], p[:])
            return s

        Sla = sum_bcast(lapool_c, T, "Sla")
        Sgo = sum_bcast(gopool_c, T, "Sgo")
        Sga = sum_bcast(gapool_c, 1, "Sga")

        pmask = cpool.tile([64, 2], FP, tag="pmask")
        nc.vector.memset(pmask[:], 0.0)
        nc.sync.dma_start(pmask[0:T, 0:1], la_pool[:][:, None])
        nc.sync.dma_start(pmask[T:2 * T, 1:2], la_pool[:][:, None])

        qembT = cpool.tile([128, ET, BL, T], FP, tag="qembT")
        wxb = cpool.tile([BL, G], FP, tag="wxb")
        QtT = persist.tile([128, ET, NH * RQ], BF, tag="QtT")
        pcxall = persist.tile([2, BL * D], FP, tag="pcxall")
        goutT = cpool.tile([128, ET, BL], FP, tag="goutT")
        aT = cpool.tile([128, ET, BL], FP, tag="aT")

        # ================= phase A: cls -> a, gout ========================
        with tc.tile_pool(name="ph0", bufs=1) as ph0:
            clsn = ph0.tile([BL, D], FP, tag="clsn")
            nc.sync.dma_start(clsn[:], img[0:BL, 0, :])
            ptr = psC.tile([128, 512], FP, tag="pd")
            for kt in range(ET):
                nc.tensor.matmul(ptr[:, 8 * kt:8 * kt + 8], clsn[:, 128 * kt:128 * (kt + 1)],
                                 ident[:BL, :BL], is_transpose=True, skip_group_check=True)
            clsT = ph0.tile([128, ET, BL], FP, tag="clsT")
            nc.vector.tensor_copy(clsT[:].rearrange("p a b -> p (a b)"), ptr[:, :8 * ET])

            def dense_T(w_nat_dram, rhsT, biasT, scaleT, otile, wtag):
                wsb = ph0.tile([128, ET, D], FP, tag=wtag)
                for c in range(ET):
                    nc.sync.dma_start(wsb[:, c, :], w_nat_dram[128 * c:128 * (c + 1), :])
                for mt in range(ET):
                    p = psC.tile([128, BL], FP, tag="pd")
                    for kt in range(ET):
                        nc.tensor.matmul(p[:], wsb[:, kt, 128 * mt:128 * (mt + 1)],
                                         rhsT[:, kt, :], start=(kt == 0), stop=(kt == ET - 1))
                    if scaleT is None:
                        nc.vector.tensor_scalar(otile[:, mt, :], p[:], biasT[:, mt:mt + 1],
                                                None, OP.add)
                    else:
                        nc.vector.tensor_scalar(otile[:, mt, :], p[:], biasT[:, mt:mt + 1],
                                                scaleT[:, 0:1], OP.add, OP.mult)

            A2T = ph0.tile([128, ET, BL], FP, tag="A2T")
            dense_T(ga_w[2], clsT, b2gaT, None, A2T, "wA")
            dense_T(ga_w[3], A2T, b3gaT, Sga, aT, "wB")
            G2T = ph0.tile([128, ET, BL], FP, tag="G2T")
            dense_T(go_w[2], clsT, b2goT, None, G2T, "wA2")
            dense_T(go_w[3], G2T, b3goT, Sgo, goutT, "wB2")

        # ================= phase B: GRU ===================================
        with tc.tile_pool(name="wbig", bufs=1) as wbig, \
             tc.tile_pool(name="wnat", bufs=4) as wnat, \
             tc.tile_pool(name="g1", bufs=1) as g1:
            bihr = g1.tile([1, G], FP, tag="bihr")
            nc.sync.dma_start(bihr[:], b_ih[:][None, :])
            bhhr = g1.tile([1, G], FP, tag="bhhr")
            nc.sync.dma_start(bhhr[:], b_hh[:][None, :])
            bhhr_r = g1.tile([1, G], FP, tag="bhhr_r")
            nc.vector.tensor_copy(_r(bhhr_r[:]), bhhr[:])
            combr = g1.tile([1, G], FP, tag="combr")
            nc.vector.tensor_copy(combr[:], bihr[:])
            nc.vector.tensor_add(combr[:, 0:2 * D], combr[:, 0:2 * D], bhhr[:, 0:2 * D])

            WT = wbig.tile([128, ET, G], FP, tag="WT")

            def build_WT(w_dram):
                jts = kchunks(G)
                for g0 in range(0, len(jts), 4):
                    grp = jts[g0:g0 + 4]
                    nats = []
                    for (j0, jw) in grp:
                        wn = wnat.tile([128, D], FP, tag="wn")
                        nc.sync.dma_start(wn[:jw, :], w_dram[j0:j0 + jw, :])
                        nats.append((wn, j0, jw))
                    for et in range(ET):
                        pt = psB.tile([128, 512], FP, tag="ptw")
                        for i, (wn, j0, jw) in enumerate(nats):
                            nc.tensor.matmul(pt[:, 128 * i:128 * i + jw],
                                             wn[:jw, 128 * et:128 * (et + 1)],
                                             ident[:jw, :jw], is_transpose=True,
                                             skip_group_check=True)
                        w0 = grp[0][0]
                        wlen = sum(jw for (_, _, jw) in nats)
                        if et % 2 == 0:
                            nc.vector.tensor_copy(_r(WT[:, et, w0:w0 + wlen]), pt[:, :wlen])
                        else:
                            nc.scalar.copy(_r(WT[:, et, w0:w0 + wlen]), pt[:, :wlen])

            build_WT(w_ih)
            for (j0, jw) in CH_G:
                p = psA.tile([BL, 512], FP, tag="wh0")
                for kt in range(ET):
                    nc.tensor.matmul(p[:, :jw], aT[:, kt, :], WT[:, kt, j0:j0 + jw].bitcast(FP),
                                     start=(kt == 0), stop=False)
                nc.tensor.matmul(p[:, :jw], ones1[:1, :BL], combr[:, j0:j0 + jw],
                                 start=False, stop=True)
                nc.vector.tensor_copy(_r(wxb[:, j0:j0 + jw]), p[:, :jw])

            build_WT(w_hh)

            hnat = gstate.tile([BL, D], FP, tag="hnat")
            nc.sync.dma_start(hnat[:], h0[:, :])
            ptr0 = psC.tile([128, 512], FP, tag="pd")
            for kt in range(ET):
                nc.tensor.matmul(ptr0[:, 8 * kt:8 * kt + 8], hnat[:, 128 * kt:128 * (kt + 1)],
                                 ident[:BL, :BL], is_transpose=True, skip_group_check=True)
            hT = gstate.tile([128, ET, BL], FP, tag="hT")
            nc.vector.tensor_copy(_r(hT[:].rearrange("p a b -> p (a b)")), ptr0[:, :8 * ET])

            emit_order = [0, 1, 3, 4, 2]
            KSTEPS = int(os.environ.get("KSTEPS", str(T)))
            for t in range(KSTEPS):
                ps = {}
                for ci in emit_order:
                    j0, jw = CH_G[ci]
                    p = psA.tile([BL, 512], FP, tag=f"wh{ci}")
                    for kt in range(ET):
                        nc.tensor.matmul(p[:, :jw], _r(hT[:, kt, :]),
                                         _r(WT[:, kt, j0:j0 + jw]),
                                         start=(kt == 0), stop=False)
                    if j0 >= 2 * D:
                        nc.tensor.matmul(p[:, :jw], _r(ones1r[:1, :BL]), _r(bhhr_r[:, j0:j0 + jw]),
                                         start=False, stop=True)
                    else:
                        nc.tensor.matmul(p[:, :jw], _r(identr[:BL, :BL]),
                                         _r(wxb[:, j0:j0 + jw]), start=False, stop=True)
                    ps[ci] = p
                r_sig = g1.tile([BL, 2 * D], FP, tag="rz")
                nc.scalar.activation(r_sig[:, 0:512], ps[0][:, 0:512], AF.Sigmoid)
                nc.scalar.activation(r_sig[:, 512:768], ps[1][:, 0:256], AF.Sigmoid)
                nc.scalar.activation(r_sig[:, 768:1024], ps[1][:, 256:512], AF.Sigmoid)
                nc.scalar.activation(r_sig[:, 1024:1536], ps[2][:, 0:512], AF.Sigmoid)
                rwn = g1.tile([BL, D], FP, tag="rwn")
                nc.vector.tensor_mul(rwn[:, 0:512], r_sig[:, 0:512], ps[3][:, 0:512])
                nc.vector.tensor_mul(rwn[:, 512:768], r_sig[:, 512:768], ps[4][:, 0:256])
                npre = g1.tile([BL, D], FP, tag="npre")
                nc.vector.tensor_add(npre[:], rwn[:], wxb[:, 2 * D:3 * D].bitcast(FP))
                nt_ = g1.tile([BL, D], FP, tag="nt")
                nc.scalar.activation(nt_[:], npre[:], AF.Tanh)
                zn = g1.tile([BL, D], FP, tag="zn")
                nc.vector.tensor_mul(zn[:], r_sig[:, 768:1536], nt_[:])
                zh = g1.tile([BL, D], FP, tag="zh")
                nc.gpsimd.tensor_mul(zh[:], r_sig[:, 768:1536], hnat[:])
                d1 = g1.tile([BL, D], FP, tag="d1")
                nc.vector.tensor_sub(d1[:], nt_[:], zn[:])
                hnat = gstate.tile([BL, D], FP, tag="hnat")
                nc.vector.tensor_add(hnat[:], d1[:], zh[:])
                ptr_t = psC.tile([128, 512], FP, tag="pd")
                for kt in range(ET):
                    nc.tensor.matmul(ptr_t[:, 8 * kt:8 * kt + 8],
                                     hnat[:, 128 * kt:128 * (kt + 1)], ident[:BL, :BL],
                                     is_transpose=True, skip_group_check=True)
                hT = gstate.tile([128, ET, BL], FP, tag="hT")
                nc.vector.tensor_copy(_r(hT[:].rearrange("p a b -> p (a b)")), ptr_t[:, :8 * ET])
                nc.scalar.copy(_r(qembT[:, :, :, t].rearrange("p a b -> p (a b)")),
                               hT[:].rearrange("p a b -> p (a b)"))
                if t < KSTEPS - 1:
                    for fi in range(int(os.environ.get("KFILL", "8"))):
                        pf = psB.tile([128, 512], FP, tag="ptw")
                        nc.tensor.matmul(pf[:], _r(identr[:]),
                                         _r(WT[:, fi % ET, 0:512]),
                                         start=True, stop=True)

        # ================= phase C: Q^T, W1^T, Qt^T =======================
        if PHASES >= 2:
          with tc.tile_pool(name="prep", bufs=1) as prep:
              W0 = prep.tile([128, ET, D], FP, tag="W0")
              for c in range(ET):
                  nc.sync.dma_start(W0[:, c, :], la_w[0][128 * c:128 * (c + 1), :])
              W0r = prep.tile([128, ET, D], FP, tag="W0r")
              nc.vector.tensor_copy(_r(W0r[:].rearrange("p a b -> p (a b)")),
                                    W0[:].rearrange("p a b -> p (a b)"))
              QT = prep.tile([128, ET, RQ], FP, tag="QT")
              qflat = qembT[:].rearrange("p a b t -> p a (b t)")
              for mt in range(ET):
                  p = psC.tile([128, RQ], FP, tag="pd")
                  for kt in range(ET):
                      nc.tensor.matmul(p[:], _r(W0r[:, kt, 128 * mt:128 * (mt + 1)]),
                                       _r(qflat[:, kt, :]), start=(kt == 0), stop=(kt == ET - 1))
                  nc.vector.tensor_scalar(_r(QT[:, mt, :]), p[:], b0laT[:, mt:mt + 1], None, OP.add)
              W1n = prep.tile([128, ET, D], FP, tag="W1n")
              for c in range(ET):
                  nc.sync.dma_start(W1n[:, c, :], la_w[1][128 * c:128 * (c + 1), :])
              W1T = prep.tile([128, ET, D], FP, tag="W1T")
              for hd in range(ET):
                  for grp in range(2):
                      pt2 = psB.tile([128, 512], FP, tag="ptw")
                      for i in range(3):
                          e2 = grp * 3 + i
                          nc.tensor.matmul(pt2[:, 128 * i:128 * (i + 1)],
                                           W1n[:, e2, 128 * hd:128 * (hd + 1)],
                                           ident[:], is_transpose=True, skip_group_check=True)
                      if grp == 0:
                          nc.vector.tensor_copy(_r(W1T[:, hd, 0:384]), pt2[:, 0:384])
                      else:
                          nc.scalar.copy(_r(W1T[:, hd, 384:768]), pt2[:, 0:384])
              scl = 1.0 / float(np.sqrt(DK))
              for h in range(NH):
                  for mt in range(ET):
                      p = psC.tile([128, RQ], FP, tag="pd")
                      for i in range(3):
                          kt = h * 3 + i
                          nc.tensor.matmul(p[:], _r(W1T[:, kt, 128 * mt:128 * (mt + 1)]),
                                           _r(QT[:, kt, :]), start=(i == 0), stop=(i == 2))
                      dst = QtT[:, mt, :].rearrange("p (b h2 t) -> p b h2 t",
                                                    h2=NH, t=T)[:, :, h, :]
                      nc.scalar.activation(dst, p[:], AF.Copy, scale=scl)

        # ================= phase D: per-b attention =======================
        if PHASES >= 3:
            with tc.tile_pool(name="xb", bufs=2) as xb, \
                 tc.tile_pool(name="ab", bufs=2) as ab:
              KC = kchunks(NK)
              for b in range(BL):
                  Xn = xb.tile([128, len(KC), D], BF, tag="Xn")
                  nc.vector.memset(Xn[:, len(KC) - 1, :], 0.0)
                  for c, (k0, kw) in enumerate(KC):
                      nc.gpsimd.dma_start(Xn[:kw, c, :], img[b, 1 + k0:1 + k0 + kw, :])
                  XT = xb.tile([128, ET, len(KC) * 128], BF, tag="XT")
                  for et in range(ET):
                      for g in range(2):
                          pt = psB.tile([128, 512], BF, tag="ptw")
                          for i in range(4):
                              c = g * 4 + i
                              nc.tensor.matmul(pt[:, 128 * i:128 * (i + 1)],
                                               Xn[:, c, 128 * et:128 * (et + 1)],
                                               identb[:], is_transpose=True,
                                               skip_group_check=True)
                          if (et + g) % 2 == 0:
                              nc.vector.tensor_copy(XT[:, et, 512 * g:512 * (g + 1)], pt[:])
                          else:
                              nc.scalar.copy(XT[:, et, 512 * g:512 * (g + 1)], pt[:])
                  att = ab.tile([64, NK], BF, tag="att")
                  zacc = ab.tile([64, 2], FP, tag="zacc")
                  for ci, (n0, nw) in enumerate(CH_NK):
                      p = psA.tile([64, 512], FP, tag=f"wh{ci}")
                      for kt in range(ET):
                          nc.tensor.matmul(p[:, :nw],
                                           QtT[:, kt, b * 2 * T:(b + 1) * 2 * T],
                                           XT[:, kt, n0:n0 + nw],
                                           start=(kt == 0), stop=(kt == ET - 1))
                      nc.scalar.activation(att[:, n0:n0 + nw], p[:, :nw], AF.Exp,
                                           accum_out=zacc[:, ci:ci + 1])
                  zs = ab.tile([64, 1], FP, tag="zs")
                  nc.vector.tensor_add(zs[:], zacc[:, 0:1], zacc[:, 1:2])
                  rz = ab.tile([64, 1], FP, tag="rz1")
                  nc.vector.reciprocal(rz[:], zs[:])
                  wm = ab.tile([64, 2], BF, tag="wm")
                  nc.vector.tensor_scalar(wm[:], pmask[:], rz[:, 0:1], None, OP.mult)
                  pa_sb = ab.tile([2, NK], BF, tag="pa_sb")
                  for ci, (n0, nw) in enumerate(CH_NK):
                      p = psA.tile([2, 512], FP, tag=f"wh{2 + ci}")
                      nc.tensor.matmul(p[:, :nw], wm[:], att[:, n0:n0 + nw],
                                       start=True, stop=True)
                      nc.vector.tensor_copy(pa_sb[:, n0:n0 + nw], p[:, :nw])
                  paT = ab.tile([128, len(KC), 2], BF, tag="paT")
                  nc.vector.memset(paT[:].rearrange("p a b -> p (a b)"), 0.0)
                  ptp = psC.tile([128, 512], BF, tag="pd")
                  for c, (k0, kw) in enumerate(KC):
                      nc.tensor.matmul(ptp[:kw, 2 * c:2 * c + 2], pa_sb[:, k0:k0 + kw],
                                       identb[:2, :2], is_transpose=True, skip_group_check=True)
                      nc.vector.tensor_copy(paT[:kw, c, :], ptp[:kw, 2 * c:2 * c + 2])
                  for ci, (n0, nw) in enumerate(CH_D):
                      p = psA.tile([2, 512], FP, tag=f"wh{4 - ci}")
                      for c in range(len(KC)):
                          nc.tensor.matmul(p[:, :nw], paT[:, c, :],
                                           Xn[:, c, n0:n0 + nw],
                                           start=(c == 0), stop=(c == len(KC) - 1))
                      nc.vector.tensor_copy(pcxall[:, b * D + n0:b * D + n0 + nw], p[:, :nw])

        # ================= phase E: projections + MLP =====================
        if PHASES >= 4:
            with tc.tile_pool(name="tail", bufs=1) as tail:
              f1 = tail.tile([128, 12, 1024], FP, tag="f1")
              for c in range(12):
                  nc.sync.dma_start(f1[:, c, :], f1_w[128 * c:128 * (c + 1), :])
              f2 = tail.tile([128, 8, 512], FP, tag="f2")
              for c in range(8):
                  nc.sync.dma_start(f2[:, c, :], f2_w[128 * c:128 * (c + 1), :])
              f3 = tail.tile([128, 4, 1024], FP, tag="f3")
              for c in range(4):
                  nc.sync.dma_start(f3[:, c, :], f3_w[128 * c:128 * (c + 1), :])
              W3 = tail.tile([128, ET, D], FP, tag="W3")
              for c in range(ET):
                  nc.sync.dma_start(W3[:, c, :], la_w[3][128 * c:128 * (c + 1), :])
              vconT = tail.tile([128, ET], FP, tag="vconT")
              for mt in range(ET):
                  p = psC.tile([128, 1], FP, tag="pd")
                  for kt in range(ET):
                      nc.tensor.matmul(p[:], W3[:, kt, 128 * mt:128 * (mt + 1)],
                                       b2laT[:, kt:kt + 1], start=(kt == 0), stop=(kt == ET - 1))
                  nc.vector.tensor_scalar(vconT[:, mt:mt + 1], p[:], b3laT[:, mt:mt + 1],
                                          Sla[:, 0:1], OP.add, OP.mult)
              pcxT = tail.tile([128, ET, 2 * BL], FP, tag="pcxT")
              ptc = psC.tile([128, 512], FP, tag="pd")
              for b2 in range(BL):
                  for kt in range(ET):
                      nc.tensor.matmul(ptc[:, 2 * (b2 * ET + kt):2 * (b2 * ET + kt) + 2],
                                       pcxall[:, b2 * D + 128 * kt:b2 * D + 128 * (kt + 1)],
                                       ident[:2, :2], is_transpose=True, skip_group_check=True)
              src_v = ptc[:, :96].rearrange("p (b a h) -> p a b h", b=BL, a=ET)
              dst_v = pcxT[:].rearrange("p a (b h) -> p a b h", h=NH)
              nc.vector.tensor_copy(dst_v, src_v)
              W2 = tail.tile([128, ET, D], FP, tag="W2")
              for c in range(ET):
                  nc.sync.dma_start(W2[:, c, :], la_w[2][128 * c:128 * (c + 1), :])
              pctxT = tail.tile([128, ET, BL], FP, tag="pctxT")
              pcv = pcxT[:].rearrange("p a (b h) -> p a b h", h=NH)
              for h in range(NH):
                  for mi in range(3):
                      mt = h * 3 + mi
                      p = psC.tile([128, BL], FP, tag="pd")
                      for kt in range(ET):
                          nc.tensor.matmul(p[:], W2[:, kt, 128 * mt:128 * (mt + 1)],
                                           pcv[:, kt, :, h], start=(kt == 0), stop=(kt == ET - 1))
                      nc.vector.tensor_copy(pctxT[:, mt, :], p[:])
              loT = tail.tile([128, ET, BL], FP, tag="loT")
              for mt in range(ET):
                  p = psC.tile([128, BL], FP, tag="pd")
                  for kt in range(ET):
                      nc.tensor.matmul(p[:], W3[:, kt, 128 * mt:128 * (mt + 1)],
                                       pctxT[:, kt, :], start=(kt == 0), stop=(kt == ET - 1))
                  nc.vector.tensor_scalar(loT[:, mt, :], p[:], vconT[:, mt:mt + 1], None, OP.add)

              y1T = tail.tile([128, 8, BL], FP, tag="y1T")
              for mt in range(8):
                  p = psC.tile([128, BL], FP, tag="pd")
                  for kt in range(12):
                      r_ = loT[:, kt, :] if kt < ET else goutT[:, kt - ET, :]
                      nc.tensor.matmul(p[:], f1[:, kt, 128 * mt:128 * (mt + 1)], r_,
                                       start=(kt == 0), stop=(kt == 11))
                  nc.vector.tensor_scalar(y1T[:, mt, :], p[:], b1fT[:, mt:mt + 1], None, OP.add)
              y2T = tail.tile([128, 4, BL], FP, tag="y2T")
              for mt in range(4):
                  p = psC.tile([128, BL], FP, tag="pd")
                  for kt in range(8):
                      nc.tensor.matmul(p[:], f2[:, kt, 128 * mt:128 * (mt + 1)],
                                       y1T[:, kt, :], start=(kt == 0), stop=(kt == 7))
                  nc.scalar.activation(y2T[:, mt, :], p[:], AF.Relu, bias=b2fT[:, mt:mt + 1])
              yT = tail.tile([128, 8, BL], FP, tag="yT")
              for mt in range(8):
                  p = psC.tile([128, BL], FP, tag="pd")
                  for kt in range(4):
                      nc.tensor.matmul(p[:], f3[:, kt, 128 * mt:128 * (mt + 1)],
                                       y2T[:, kt, :], start=(kt == 0), stop=(kt == 3))
                  nc.vector.tensor_scalar(yT[:, mt, :], p[:], b3fT[:, mt:mt + 1], None, OP.add)
              ynat = tail.tile([BL, 1024], FP, tag="ynat")
              for g in range(2):
                  po = psB.tile([128, 512], FP, tag="ptw")
                  for i in range(4):
                      mt = g * 4 + i
                      nc.tensor.matmul(po[:BL, 128 * i:128 * (i + 1)], yT[:, mt, :],
                                       ident[:128, :128], is_transpose=True,
                                       skip_group_check=True)
                  nc.vector.tensor_copy(ynat[:, 512 * g:512 * (g + 1)], po[:BL, :])
              nc.sync.dma_start(out_d[:, :], ynat[:])

    nc.compile()
    return nc


_NC = None


def make_in_maps(inputs):
    B = inputs["image_local_embeds"].shape[0]
    per = B // NCORES
    in_maps = []
    for c in range(NCORES):
        sl = slice(c * per, (c + 1) * per)
        m = {
            "img": np.ascontiguousarray(np.asarray(inputs["image_local_embeds"])[sl], dtype=np.float32),
            "h0": np.ascontiguousarray(np.asarray(inputs["h0"])[sl], dtype=np.float32),
        }
        for k in ["gru_w_ih", "gru_w_hh", "gru_b_ih", "gru_b_hh", "ga_w", "ga_b",
                  "ga_pool", "la_w", "la_b", "la_pool", "go_w", "go_b", "go_pool",
                  "f1_w", "f1_b", "f2_w", "f2_b", "f3_w", "f3_b"]:
            m[k] = np.ascontiguousarray(np.asarray(inputs[k], dtype=np.float32))
        in_maps.append(m)
    return in_maps


def kernel(**inputs):
    global _NC
    if _NC is None:
        _NC = build()
    in_maps = make_in_maps(inputs)
    res = run_bass_kernel_spmd(_NC, in_maps, core_ids=list(range(NCORES)))
    return np.concatenate([res.results[c]["out"] for c in range(NCORES)], axis=0)



# revision 6
# speedup vs baseline: 1.8773x; 1.8773x over previous
"""Trainium2 Bass kernel for nn_BiVision_VQA2 (B=64,T=32,D=768,N=901).

Data-parallel over batch: 8 batch elems per core x 8 cores.

Math simplifications (validated vs reference, numpy sim rel err ~4e-3):
  - ga/go attention have a single key token -> softmax==1 -> linear in cls;
    the (cls@W2+b2)@W3+b3 chains collapse to cls@M + c with M,c precomputed
    on the HOST.  The GRU input is constant over time, so its contribution
    wx = cls@(Mga@W_ih^T) + cw is one matmul.
  - local attention: scores = (qemb@W0+b0) @ W1_h^T / sqrt(dk) @ X^T with
    row-constant terms dropped; query pooling applied to the attention
    matrix before the @X contraction; value/out projections collapse to
    per-head M2 = W2_h@W3_h (host).
Precision strategy (validated in numpy, rel err 4.1e-3 < 2e-2):
  - GRU weights + hidden state in fp8e4m3 (x16 / x1 scaling), matmuls use
    DoubleRow perf mode (2x PE throughput).  Scores path fp8 (Qt x8, X x4).
  - Everything else bf16; host pre-transposes/packs all weights so the
    device does no weight reshaping.
"""

import os
import numpy as np
import ml_dtypes
from contextlib import ExitStack

import concourse.bass as bass
import concourse.tile as tile
from concourse import bacc, mybir
from concourse.bass_utils import run_bass_kernel_spmd
from concourse.masks import make_identity

FP = mybir.dt.float32
BF = mybir.dt.bfloat16
F8 = mybir.dt.float8e4
OP = mybir.AluOpType
AF = mybir.ActivationFunctionType
DR = mybir.MatmulPerfMode.DoubleRow

NCORES = 8
BL = 8
D = 768
T = 32
G = 3 * D
NK = 900
NH = 2
DK = 384
ET = D // 128
SG = 16.0
SQ = 8.0
SX = 4.0
ESC = 1.0 / (SQ * SX * float(np.sqrt(DK)))

CH_G = [(0, 512), (512, 512), (1024, 512), (1536, 512), (2048, 256)]
CH_NK = [(0, 512), (512, 388)]
CH_D = [(0, 512), (512, 256)]
KC = [(k, min(128, NK - k)) for k in range(0, NK, 128)]

KSTEPS = int(os.environ.get("KSTEPS", str(T)))
PHASES = int(os.environ.get("KPHASES", "4"))


def build():
    nc = bacc.Bacc("TRN2", target_bir_lowering=False, debug=False,
                   enable_asserts=False)

    clsT8_d = nc.dram_tensor("clsT8", [128, ET, 16], F8, kind="ExternalInput").ap()
    clsTb_d = nc.dram_tensor("clsTb", [128, ET, 16], BF, kind="ExternalInput").ap()
    mw8_d = nc.dram_tensor("mw8", [128, ET, G], F8, kind="ExternalInput").ap()
    cw8_d = nc.dram_tensor("cw8", [1, G], F8, kind="ExternalInput").ap()
    idext8_d = nc.dram_tensor("idext8", [128, 2, 16], F8, kind="ExternalInput").ap()
    extn8_d = nc.dram_tensor("extn8", [BL, D], F8, kind="ExternalInput").ap()
    wh8_d = nc.dram_tensor("wh8", [128, ET, G], F8, kind="ExternalInput").ap()
    h0t8_d = nc.dram_tensor("h0t8", [128, ET, 16], F8, kind="ExternalInput").ap()
    h0b_d = nc.dram_tensor("h0b", [BL, D], BF, kind="ExternalInput").ap()
    mg_d = nc.dram_tensor("mg", [128, ET, D], BF, kind="ExternalInput").ap()
    cgcol_d = nc.dram_tensor("cgcol", [128, ET], FP, kind="ExternalInput").ap()
    w0_d = nc.dram_tensor("w0", [128, ET, D], BF, kind="ExternalInput").ap()
    b0col_d = nc.dram_tensor("b0col", [128, ET], FP, kind="ExternalInput").ap()
    w1t_d = nc.dram_tensor("w1t", [128, ET, D], BF, kind="ExternalInput").ap()
    pmask_d = nc.dram_tensor("pmaskb", [64, 2], BF, kind="ExternalInput").ap()
    xn_d = nc.dram_tensor("xn", [BL, NK, D], BF, kind="ExternalInput").ap()
    xt8_d = nc.dram_tensor("xt8", [BL, D, 912], F8, kind="ExternalInput").ap()
    m2_d = nc.dram_tensor("m2", [128, 2 * ET, D], BF, kind="ExternalInput").ap()
    vconcol_d = nc.dram_tensor("vconcol", [128, ET], FP, kind="ExternalInput").ap()
    f1_d = nc.dram_tensor("f1", [128, 12, 1024], BF, kind="ExternalInput").ap()
    b1row_d = nc.dram_tensor("b1row", [1, 1024], BF, kind="ExternalInput").ap()
    f2_d = nc.dram_tensor("f2", [128, 8, 512], BF, kind="ExternalInput").ap()
    b2row_d = nc.dram_tensor("b2row", [1, 512], BF, kind="ExternalInput").ap()
    f3_d = nc.dram_tensor("f3", [128, 4, 1024], BF, kind="ExternalInput").ap()
    b3row_d = nc.dram_tensor("b3row", [1, 1024], BF, kind="ExternalInput").ap()
    out_d = nc.dram_tensor("out", [BL, 1024], FP, kind="ExternalOutput").ap()

    with tile.TileContext(nc) as tc, ExitStack() as ctx:
        cpool = ctx.enter_context(tc.tile_pool(name="const", bufs=1))
        gst = ctx.enter_context(tc.tile_pool(name="gst", bufs=2))
        g1 = ctx.enter_context(tc.tile_pool(name="g1", bufs=2))
        psT = ctx.enter_context(tc.tile_pool(name="psT", bufs=2, space="PSUM"))
        psG_cm = tc.tile_pool(name="psG", bufs=1, space="PSUM")
        psG = psG_cm.__enter__()

        identf = cpool.tile([128, 128], FP, tag="identf")
        make_identity(nc, identf[:])
        identb = cpool.tile([128, 128], BF, tag="identb")
        nc.vector.tensor_copy(identb[:], identf[:])
        ones8 = cpool.tile([1, 16], F8, tag="ones8")
        nc.vector.memset(ones8[:], 1.0)
        onesb = cpool.tile([1, 16], BF, tag="onesb")
        nc.vector.memset(onesb[:], 1.0)

        wh8 = cpool.tile([128, ET, G], F8, tag="wh8")
        for tt in range(ET):
            nc.sync.dma_start(wh8[:, tt, :], wh8_d[:, tt, :])
        idext8 = cpool.tile([128, 2, 16], F8, tag="idext8")
        nc.sync.dma_start(idext8[:], idext8_d[:])
        clsTb = cpool.tile([128, ET, 16], BF, tag="clsTb")
        nc.sync.dma_start(clsTb[:], clsTb_d[:])

        qembT = cpool.tile([128, ET, BL, T], BF, tag="qembT")
        goutT = cpool.tile([128, ET, BL], BF, tag="goutT")
        ext8 = cpool.tile([128, 2, G], F8, tag="ext8")
        nc.vector.memset(ext8[:].rearrange("p a b -> p (a b)"), 0.0)
        nc.sync.dma_start(ext8[0:BL, 0, 2 * D:3 * D], extn8_d[:])
        wxn_sb = cpool.tile([BL, D], BF, tag="wxn_sb")

        # ================= phase A: wx = cls@MW + cw ======================
        with tc.tile_pool(name="phA", bufs=1) as phA:
            clsT8 = phA.tile([128, ET, 16], F8, tag="clsT8")
            nc.sync.dma_start(clsT8[:], clsT8_d[:])
            mw8 = phA.tile([128, ET, G], F8, tag="mw8")
            for tt in range(ET):
                nc.sync.dma_start(mw8[:, tt, :], mw8_d[:, tt, :])
            cw8 = phA.tile([1, G], F8, tag="cw8")
            nc.sync.dma_start(cw8[:], cw8_d[:])
            for ci, (j0, jw) in enumerate(CH_G):
                p = psG.tile([16, 512], FP, tag=f"g{ci}")
                for g in range(3):
                    nc.tensor.matmul(p[:, :jw], clsT8[:, 2 * g:2 * g + 2, :],
                                     mw8[:, 2 * g:2 * g + 2, j0:j0 + jw],
                                     start=(g == 0), stop=False, perf_mode=DR)
                nc.tensor.matmul(p[:, :jw], ones8[:1, :], cw8[:, j0:j0 + jw],
                                 start=False, stop=True)
                if j0 < 1536:
                    nc.scalar.activation(ext8[0:BL, 0, j0:j0 + jw], p[:BL, :jw],
                                         AF.Copy)
                else:
                    nc.scalar.activation(wxn_sb[:, j0 - 1536:j0 - 1536 + jw],
                                         p[:BL, :jw], AF.Copy)

        # ================= phase B: GRU ===================================
        hT8 = gst.tile([128, ET, 16], F8, tag="hT8")
        nc.sync.dma_start(hT8[:], h0t8_d[:])
        hnat = gst.tile([BL, D], BF, tag="hnat")
        nc.sync.dma_start(hnat[:], h0b_d[:])

        for t in range(KSTEPS):
            ps = {}
            for ci, (j0, jw) in enumerate(CH_G):
                p = psG.tile([16, 512], FP, tag=f"g{ci}")
                for g in range(3):
                    nc.tensor.matmul(p[:, :jw], hT8[:, 2 * g:2 * g + 2, :],
                                     wh8[:, 2 * g:2 * g + 2, j0:j0 + jw],
                                     start=(g == 0), stop=False, perf_mode=DR)
                nc.tensor.matmul(p[:, :jw], idext8[:], ext8[:, :, j0:j0 + jw],
                                 start=False, stop=True, perf_mode=DR)
                ps[ci] = p
            rz = g1.tile([BL, 2 * D], BF, tag="rz")
            nc.scalar.activation(rz[:, 0:512], ps[0][:BL, :], AF.Sigmoid,
                                 scale=1.0 / SG)
            nc.scalar.activation(rz[:, 512:1024], ps[1][:BL, :], AF.Sigmoid,
                                 scale=1.0 / SG)
            nc.scalar.activation(rz[:, 1024:1536], ps[2][:BL, :], AF.Sigmoid,
                                 scale=1.0 / SG)
            nt_ = g1.tile([BL, D], BF, tag="nt")
            hnew = gst.tile([BL, D], BF, tag="hnat")
            for ci, f0, fw in [(3, 0, 512), (4, 512, 256)]:
                t1 = g1.tile([BL, 512], BF, tag=f"t1_{ci}")
                nc.vector.tensor_mul(t1[:, :fw], rz[:, f0:f0 + fw],
                                     ps[ci][:BL, :fw])
                t2 = g1.tile([BL, 512], BF, tag=f"t2_{ci}")
                nc.vector.tensor_add(t2[:, :fw], t1[:, :fw],
                                     wxn_sb[:, f0:f0 + fw])
                nc.scalar.activation(nt_[:, f0:f0 + fw], t2[:, :fw], AF.Tanh,
                                     scale=1.0 / SG)
                d_ = g1.tile([BL, 512], BF, tag=f"d_{ci}")
                nc.vector.tensor_sub(d_[:, :fw], hnat[:, f0:f0 + fw],
                                     nt_[:, f0:f0 + fw])
                e_ = g1.tile([BL, 512], BF, tag=f"e_{ci}")
                nc.vector.tensor_mul(e_[:, :fw], rz[:, D + f0:D + f0 + fw],
                                     d_[:, :fw])
                nc.vector.tensor_add(hnew[:, f0:f0 + fw], nt_[:, f0:f0 + fw],
                                     e_[:, :fw])
            pt = psT.tile([128, 64], BF, tag="pt", padded_shape=[128, 1024])
            for kt in range(ET):
                nc.tensor.matmul(pt[:, 8 * kt:8 * kt + 8],
                                 hnew[:, 128 * kt:128 * (kt + 1)],
                                 identb[:BL, :BL], is_transpose=True,
                                 skip_group_check=True)
            hT8 = gst.tile([128, ET, 16], F8, tag="hT8")
            nc.vector.memset(hT8[:].rearrange("p a b -> p (a b)"), 0.0)
            nc.vector.tensor_copy(
                hT8[:, :, 0:BL], pt[:, :48].rearrange("p (a b) -> p a b", b=BL))
            nc.scalar.copy(qembT[:, :, :, t].rearrange("p a b -> p (a b)"),
                           pt[:, :48])
            hnat = hnew

        psG_cm.__exit__(None, None, None)

        # ================= gout = cls@Mg + cg (feature-major) =============
        with tc.tile_pool(name="phG", bufs=1) as phG, \
             tc.tile_pool(name="psC", bufs=2, space="PSUM") as psC:
            mg = phG.tile([128, ET, D], BF, tag="mg")
            for tt in range(ET):
                nc.sync.dma_start(mg[:, tt, :], mg_d[:, tt, :])
            cgcol = phG.tile([128, ET], FP, tag="cgcol")
            nc.sync.dma_start(cgcol[:], cgcol_d[:])
            for mt in range(ET):
                p = psC.tile([128, BL], FP, tag="pg", padded_shape=[128, 512])
                for kt in range(ET):
                    nc.tensor.matmul(p[:], mg[:, kt, 128 * mt:128 * (mt + 1)],
                                     clsTb[:, kt, 0:BL],
                                     start=(kt == 0), stop=(kt == ET - 1))
                nc.vector.tensor_scalar(goutT[:, mt, :], p[:],
                                        cgcol[:, mt:mt + 1], None, OP.add)

        # ================= phase C: QT, QtT8 ==============================
        QtT8 = cpool.tile([128, ET, 512], F8, tag="QtT8")
        if PHASES >= 2:
            with tc.tile_pool(name="phC", bufs=1) as phC, \
                 tc.tile_pool(name="psC", bufs=2, space="PSUM") as psC:
                w0 = phC.tile([128, ET, D], BF, tag="w0")
                for tt in range(ET):
                    nc.sync.dma_start(w0[:, tt, :], w0_d[:, tt, :])
                b0col = phC.tile([128, ET], FP, tag="b0col")
                nc.sync.dma_start(b0col[:], b0col_d[:])
                w1tt = phC.tile([128, ET, D], BF, tag="w1t")
                for tt in range(ET):
                    nc.sync.dma_start(w1tt[:, tt, :], w1t_d[:, tt, :])
                qflat = qembT[:].rearrange("p a b t -> p a (b t)")
                QT = phC.tile([128, ET, BL * T], BF, tag="QT")
                for mt in range(ET):
                    p = psC.tile([128, BL * T], FP, tag="pc", padded_shape=[128, 512])
                    for kt in range(ET):
                        nc.tensor.matmul(p[:], w0[:, kt, 128 * mt:128 * (mt + 1)],
                                         qflat[:, kt, :],
                                         start=(kt == 0), stop=(kt == ET - 1))
                    nc.vector.tensor_scalar(QT[:, mt, :], p[:],
                                            b0col[:, mt:mt + 1], None, OP.add)
                for hd in range(NH):
                    for mt in range(ET):
                        p = psC.tile([128, BL * T], FP, tag="pc", padded_shape=[128, 512])
                        for i in range(3):
                            kt = 3 * hd + i
                            nc.tensor.matmul(p[:],
                                             w1tt[:, kt, 128 * mt:128 * (mt + 1)],
                                             QT[:, kt, :],
                                             start=(i == 0), stop=(i == 2))
                        dst = QtT8[:, mt, :].rearrange(
                            "p (b h2 t) -> p b h2 t", h2=NH, t=T)[:, :, hd, :]
                        src = p[:].rearrange("p (b t) -> p b t", t=T)
                        nc.scalar.activation(dst, src, AF.Copy, scale=SQ)

        # ================= phase D: per-b attention =======================
        pcxT = cpool.tile([128, ET, 2 * BL], BF, tag="pcxT")
        if PHASES >= 3:
            with tc.tile_pool(name="xb", bufs=3) as xb, \
                 tc.tile_pool(name="ab", bufs=2) as ab, \
                 tc.tile_pool(name="psS", bufs=2, space="PSUM") as psS:
                pmaskb = cpool.tile([64, 2], BF, tag="pmaskb")
                nc.sync.dma_start(pmaskb[:], pmask_d[:])
                for b in range(BL):
                    xn_t = xb.tile([128, 8, D], BF, tag="xn")
                    nc.vector.memset(xn_t[:, 7, :], 0.0)
                    for c, (k0, kw) in enumerate(KC):
                        nc.gpsimd.dma_start(xn_t[:kw, c, :],
                                            xn_d[b, k0:k0 + kw, :])
                    xt_t = xb.tile([128, ET, 912], F8, tag="xt")
                    for tt in range(ET):
                        nc.gpsimd.dma_start(xt_t[:, tt, :],
                                            xt8_d[b, 128 * tt:128 * (tt + 1), :])
                    att = ab.tile([64, NK], BF, tag="att")
                    zacc = ab.tile([64, 2], FP, tag="zacc")
                    for ci, (n0, nw) in enumerate(CH_NK):
                        p = psS.tile([64, 512], FP, tag="s")
                        for g in range(3):
                            nc.tensor.matmul(
                                p[:, :nw],
                                QtT8[:, 2 * g:2 * g + 2, 64 * b:64 * b + 64],
                                xt_t[:, 2 * g:2 * g + 2, n0:n0 + nw],
                                start=(g == 0), stop=(g == 2), perf_mode=DR)
                        nc.scalar.activation(att[:, n0:n0 + nw], p[:, :nw],
                                             AF.Exp, scale=ESC,
                                             accum_out=zacc[:, ci:ci + 1])
                    zs = ab.tile([64, 1], FP, tag="zs")
                    nc.vector.tensor_add(zs[:], zacc[:, 0:1], zacc[:, 1:2])
                    rcp = ab.tile([64, 1], FP, tag="rcp")
                    nc.vector.reciprocal(rcp[:], zs[:])
                    wm = ab.tile([64, 2], BF, tag="wm")
                    nc.vector.tensor_scalar(wm[:], pmaskb[:], rcp[:, 0:1],
                                            None, OP.mult)
                    pa_sb = ab.tile([2, NK], BF, tag="pa_sb")
                    for ci, (n0, nw) in enumerate(CH_NK):
                        p2 = psS.tile([2, 512], FP, tag="p2")
                        nc.tensor.matmul(p2[:, :nw], wm[:], att[:, n0:n0 + nw],
                                         start=True, stop=True)
                        nc.vector.tensor_copy(pa_sb[:, n0:n0 + nw], p2[:, :nw])
                    paT = ab.tile([128, len(KC), 2], BF, tag="paT")
                    nc.vector.memset(paT[:].rearrange("p a b -> p (a b)"), 0.0)
                    ptp = psT.tile([128, 16], BF, tag="pt", padded_shape=[128, 1024])
                    for c, (k0, kw) in enumerate(KC):
                        nc.tensor.matmul(ptp[:kw, 2 * c:2 * c + 2],
                                         pa_sb[:, k0:k0 + kw], identb[:2, :2],
                                         is_transpose=True,
                                         skip_group_check=True)
                        nc.vector.tensor_copy(paT[:kw, c, :],
                                              ptp[:kw, 2 * c:2 * c + 2])
                    pcx = ab.tile([2, D], BF, tag="pcx")
                    for ci, (n0, nw) in enumerate(CH_D):
                        p = psS.tile([2, 512], FP, tag="v")
                        for c in range(len(KC)):
                            nc.tensor.matmul(p[:, :nw], paT[:, c, :],
                                             xn_t[:, c, n0:n0 + nw],
                                             start=(c == 0),
                                             stop=(c == len(KC) - 1))
                        nc.vector.tensor_copy(pcx[:, n0:n0 + nw], p[:, :nw])
                    ptc = psT.tile([128, 16], BF, tag="pt", padded_shape=[128, 1024])
                    for kt in range(ET):
                        nc.tensor.matmul(ptc[:, 2 * kt:2 * kt + 2],
                                         pcx[:, 128 * kt:128 * (kt + 1)],
                                         identb[:2, :2], is_transpose=True,
                                         skip_group_check=True)
                    nc.vector.tensor_copy(
                        pcxT[:, :, 2 * b:2 * b + 2],
                        ptc[:, :2 * ET].rearrange("p (a c) -> p a c", c=2))

        # ================= phase E: projections + MLP =====================
        if PHASES >= 4:
            with tc.tile_pool(name="tail", bufs=1) as tail, \
                 tc.tile_pool(name="psE", bufs=2, space="PSUM") as psE:
                m2 = tail.tile([128, 2 * ET, D], BF, tag="m2")
                for tt in range(2 * ET):
                    nc.sync.dma_start(m2[:, tt, :], m2_d[:, tt, :])
                vconcol = tail.tile([128, ET], FP, tag="vconcol")
                nc.sync.dma_start(vconcol[:], vconcol_d[:])
                f1 = tail.tile([128, 12, 1024], BF, tag="f1")
                for tt in range(12):
                    nc.sync.dma_start(f1[:, tt, :], f1_d[:, tt, :])
                f2 = tail.tile([128, 8, 512], BF, tag="f2")
                for tt in range(8):
                    nc.sync.dma_start(f2[:, tt, :], f2_d[:, tt, :])
                f3 = tail.tile([128, 4, 1024], BF, tag="f3")
                for tt in range(4):
                    nc.sync.dma_start(f3[:, tt, :], f3_d[:, tt, :])
                b1row = tail.tile([1, 1024], BF, tag="b1row")
                nc.sync.dma_start(b1row[:], b1row_d[:])
                b2row = tail.tile([1, 512], BF, tag="b2row")
                nc.sync.dma_start(b2row[:], b2row_d[:])
                b3row = tail.tile([1, 1024], BF, tag="b3row")
                nc.sync.dma_start(b3row[:], b3row_d[:])

                loT = tail.tile([128, ET, BL], BF, tag="loT")
                for mt in range(ET):
                    p = psE.tile([128, BL], FP, tag="pe", padded_shape=[128, 512])
                    nmm = 2 * ET
                    k = 0
                    for hd in range(NH):
                        for kt in range(ET):
                            pcv = pcxT[:].rearrange(
                                "p a (b h) -> p a b h", h=2)[:, kt, :, hd]
                            nc.tensor.matmul(
                                p[:], m2[:, ET * hd + kt, 128 * mt:128 * (mt + 1)],
                                pcv,
                                start=(k == 0), stop=(k == nmm - 1))
                            k += 1
                    nc.vector.tensor_scalar(loT[:, mt, :], p[:],
                                            vconcol[:, mt:mt + 1], None, OP.add)

                y1b = tail.tile([BL, 1024], BF, tag="y1b")
                for ch in range(2):
                    p = psE.tile([BL, 512], FP, tag="pe")
                    for kt in range(12):
                        lhs = loT[:, kt, :] if kt < ET else goutT[:, kt - ET, :]
                        nc.tensor.matmul(p[:], lhs,
                                         f1[:, kt, 512 * ch:512 * (ch + 1)],
                                         start=(kt == 0), stop=False)
                    nc.tensor.matmul(p[:], onesb[:1, :BL],
                                     b1row[:, 512 * ch:512 * (ch + 1)],
                                     start=False, stop=True)
                    nc.scalar.activation(y1b[:, 512 * ch:512 * (ch + 1)], p[:],
                                         AF.Copy)
                pt1 = psT.tile([128, 64], BF, tag="pt", padded_shape=[128, 1024])
                for kt in range(8):
                    nc.tensor.matmul(pt1[:, 8 * kt:8 * kt + 8],
                                     y1b[:, 128 * kt:128 * (kt + 1)],
                                     identb[:BL, :BL], is_transpose=True,
                                     skip_group_check=True)
                y1T = tail.tile([128, 8, BL], BF, tag="y1T")
                nc.vector.tensor_copy(y1T[:].rearrange("p a b -> p (a b)"),
                                      pt1[:, :64])

                p = psE.tile([BL, 512], FP, tag="pe")
                for kt in range(8):
                    nc.tensor.matmul(p[:], y1T[:, kt, :], f2[:, kt, :],
                                     start=(kt == 0), stop=False)
                nc.tensor.matmul(p[:], onesb[:1, :BL], b2row[:],
                                 start=False, stop=True)
                y2b = tail.tile([BL, 512], BF, tag="y2b")
                nc.scalar.activation(y2b[:], p[:], AF.Relu)
                pt2 = psT.tile([128, 32], BF, tag="pt", padded_shape=[128, 1024])
                for kt in range(4):
                    nc.tensor.matmul(pt2[:, 8 * kt:8 * kt + 8],
                                     y2b[:, 128 * kt:128 * (kt + 1)],
                                     identb[:BL, :BL], is_transpose=True,
                                     skip_group_check=True)
                y2T = tail.tile([128, 4, BL], BF, tag="y2T")
                nc.vector.tensor_copy(y2T[:].rearrange("p a b -> p (a b)"),
                                      pt2[:, :32])

                ynat = tail.tile([BL, 1024], FP, tag="ynat")
                for ch in range(2):
                    p = psE.tile([BL, 512], FP, tag="pe")
                    for kt in range(4):
                        nc.tensor.matmul(p[:], y2T[:, kt, :],
                                         f3[:, kt, 512 * ch:512 * (ch + 1)],
                                         start=(kt == 0), stop=False)
                    nc.tensor.matmul(p[:], onesb[:1, :BL],
                                     b3row[:, 512 * ch:512 * (ch + 1)],
                                     start=False, stop=True)
                    nc.vector.tensor_copy(ynat[:, 512 * ch:512 * (ch + 1)],
                                          p[:])
                nc.sync.dma_start(out_d[:, :], ynat[:])

    nc.compile()
    return nc


_NC = None


def _bf(x):
    return np.ascontiguousarray(x).astype(ml_dtypes.bfloat16)


def _f8(x):
    return np.ascontiguousarray(x).astype(ml_dtypes.float8_e4m3)


def _tile6(w):
    """[768, J] -> [128, 6, J] with [p, t, j] = w[128t+p, j]"""
    J = w.shape[1]
    return np.ascontiguousarray(w.reshape(ET, 128, J).transpose(1, 0, 2))


def make_in_maps(inputs):
    f32 = np.float32
    img = np.asarray(inputs["image_local_embeds"], f32)
    h0 = np.asarray(inputs["h0"], f32)
    w_ih = np.asarray(inputs["gru_w_ih"], f32)
    w_hh = np.asarray(inputs["gru_w_hh"], f32)
    b_ih = np.asarray(inputs["gru_b_ih"], f32)
    b_hh = np.asarray(inputs["gru_b_hh"], f32)
    ga_w = np.asarray(inputs["ga_w"], f32)
    ga_b = np.asarray(inputs["ga_b"], f32)
    ga_pool = np.asarray(inputs["ga_pool"], f32)
    la_w = np.asarray(inputs["la_w"], f32)
    la_b = np.asarray(inputs["la_b"], f32)
    la_pool = np.asarray(inputs["la_pool"], f32)
    go_w = np.asarray(inputs["go_w"], f32)
    go_b = np.asarray(inputs["go_b"], f32)
    go_pool = np.asarray(inputs["go_pool"], f32)
    f1_w = np.asarray(inputs["f1_w"], f32)
    f1_b = np.asarray(inputs["f1_b"], f32)
    f2_w = np.asarray(inputs["f2_w"], f32)
    f2_b = np.asarray(inputs["f2_b"], f32)
    f3_w = np.asarray(inputs["f3_w"], f32)
    f3_b = np.asarray(inputs["f3_b"], f32)

    Mga = ga_pool[0] * (ga_w[2] @ ga_w[3])
    cga = ga_pool[0] * (ga_b[2] @ ga_w[3] + ga_b[3])
    MW = Mga @ w_ih.T
    cw = cga @ w_ih.T + b_ih
    cw[:2 * D] += b_hh[:2 * D]
    Sgo = go_pool.sum()
    Mg = Sgo * (go_w[2] @ go_w[3])
    cg = Sgo * (go_b[2] @ go_w[3] + go_b[3])
    Sla = la_pool.sum()
    M2 = np.stack([la_w[2][:, hd * DK:(hd + 1) * DK]
                   @ la_w[3][hd * DK:(hd + 1) * DK, :] for hd in range(NH)])
    vcon = Sla * (la_b[2] @ la_w[3] + la_b[3])
    W1T = np.ascontiguousarray(la_w[1].T)

    idext = np.zeros((128, 2, 16), f32)
    for b in range(BL):
        idext[b, 0, b] = 1.0
    pmask = np.zeros((64, 2), f32)
    pmask[0:T, 0] = la_pool
    pmask[T:2 * T, 1] = la_pool

    mw8 = _f8(_tile6(SG * MW))
    cw8 = _f8((SG * cw)[None, :])
    wh8 = _f8(_tile6(SG * w_hh.T))
    extn8 = _f8(np.broadcast_to(SG * b_hh[2 * D:], (BL, D)).copy())
    idext8 = _f8(idext)
    mg = _bf(_tile6(Mg))
    cgcol = np.ascontiguousarray(cg.reshape(ET, 128).T)
    w0 = _bf(_tile6(la_w[0]))
    b0col = np.ascontiguousarray(la_b[0].reshape(ET, 128).T)
    w1t = _bf(_tile6(W1T))
    m2 = _bf(np.concatenate(
        [_tile6(M2[0]), _tile6(M2[1])], axis=1))
    vconcol = np.ascontiguousarray(vcon.reshape(ET, 128).T)
    f1p = _bf(f1_w.reshape(12, 128, 1024).transpose(1, 0, 2))
    f2p = _bf(f2_w.reshape(8, 128, 512).transpose(1, 0, 2))
    f3p = _bf(f3_w.reshape(4, 128, 1024).transpose(1, 0, 2))

    in_maps = []
    B = img.shape[0]
    per = B // NCORES
    for c in range(NCORES):
        sl = slice(c * per, (c + 1) * per)
        cls = img[sl, 0, :]                     # [8, 768]
        X = img[sl, 1:, :]                      # [8, 900, 768]
        clsT = np.zeros((128, ET, 16), f32)
        clsT[:, :, :BL] = cls.T.reshape(ET, 128, BL).transpose(1, 0, 2)
        h0c = h0[sl]
        h0t = np.zeros((128, ET, 16), f32)
        h0t[:, :, :BL] = h0c.T.reshape(ET, 128, BL).transpose(1, 0, 2)
        xt = np.zeros((per, D, 912), f32)
        xt[:, :, :NK] = SX * X.transpose(0, 2, 1)
        m = {
            "clsT8": _f8(clsT),
            "clsTb": _bf(clsT),
            "mw8": mw8, "cw8": cw8, "idext8": idext8, "extn8": extn8,
            "wh8": wh8,
            "h0t8": _f8(h0t), "h0b": _bf(h0c),
            "mg": mg, "cgcol": cgcol.astype(f32),
            "w0": w0, "b0col": b0col.astype(f32),
            "w1t": w1t, "pmaskb": _bf(pmask),
            "xn": _bf(X), "xt8": _f8(xt),
            "m2": m2, "vconcol": vconcol.astype(f32),
            "f1": f1p, "b1row": _bf(f1_b[None, :]),
            "f2": f2p, "b2row": _bf(f2_b[None, :]),
            "f3": f3p, "b3row": _bf(f3_b[None, :]),
        }
        in_maps.append(m)
    return in_maps


def kernel(**inputs):
    global _NC
    if _NC is None:
        _NC = build()
    in_maps = make_in_maps(inputs)
    res = run_bass_kernel_spmd(_NC, in_maps, core_ids=list(range(NCORES)))
    return np.concatenate([res.results[c]["out"] for c in range(NCORES)],
                          axis=0)


# revision 11
# speedup vs baseline: 2.2226x; 1.1839x over previous
"""Trainium2 Bass kernel for nn_BiVision_VQA2 (B=64,T=32,D=768,N=901).

Data-parallel over batch: 8 batch elems per core x 8 cores.

Math simplifications (validated vs reference, numpy sim rel err ~4e-3):
  - ga/go attention have a single key token -> softmax==1 -> linear in cls;
    the (cls@W2+b2)@W3+b3 chains collapse to cls@M + c with M,c precomputed
    on the HOST.  The GRU input is constant over time, so its contribution
    wx = cls@(Mga@W_ih^T) + cw is one matmul.
  - local attention: scores = (qemb@W0+b0) @ W1_h^T / sqrt(dk) @ X^T with
    row-constant terms dropped; query pooling applied to the attention
    matrix before the @X contraction; value/out projections collapse to
    per-head M2 = W2_h@W3_h (host).
Precision strategy (validated in numpy, rel err 4.1e-3 < 2e-2):
  - GRU weights + hidden state in fp8e4m3 (x16 / x1 scaling), matmuls use
    DoubleRow perf mode.  Scores path fp8 (Qt x8, X x4).
  - Everything else bf16; host pre-transposes/packs all weights so the
    device does no weight reshaping.
Scheduling:
  - all weight DMAs emitted up-front (stream during the GRU)
  - GRU: merged psum tiles (r|z in one 3-bank tile), split sigmoids so
    they overlap the weight stream, h' = (1-z)*n + z*h with z-terms
    precomputed, n-path split so only the 2nd half is latency-exposed
  - phase D software-pipelined: softmax/value tail of batch b-1 is
    emitted after the score matmuls of batch b so the PE never stalls
  - occasional dummy matmuls keep the PE HAM clock-gate at 2.4 GHz
"""

import os
import numpy as np
import ml_dtypes
from contextlib import ExitStack

import concourse.bass as bass
import concourse.tile as tile
from concourse import bacc, mybir
from concourse.bass_utils import run_bass_kernel_spmd
from concourse.masks import make_identity

FP = mybir.dt.float32
BF = mybir.dt.bfloat16
F8 = mybir.dt.float8e4
OP = mybir.AluOpType
AF = mybir.ActivationFunctionType
DR = mybir.MatmulPerfMode.DoubleRow

NCORES = 8
BL = 8
D = 768
T = 32
G = 3 * D
NK = 900
NH = 2
DK = 384
ET = D // 128
SG = 16.0
SQ = 8.0
SX = 4.0
ESC = 1.0 / (SQ * SX * float(np.sqrt(DK)))

CH_NK = [(0, 512), (512, 388)]
CH_D = [(0, 512), (512, 256)]
KC = [(k, min(128, NK - k)) for k in range(0, NK, 128)]

KSTEPS = int(os.environ.get("KSTEPS", str(T)))
PHASES = int(os.environ.get("KPHASES", "4"))
NDUM = int(os.environ.get("KDUM", "2"))


def build():
    nc = bacc.Bacc("TRN2", target_bir_lowering=False, debug=False,
                   enable_asserts=False)

    clsT8_d = nc.dram_tensor("clsT8", [128, ET, 16], F8, kind="ExternalInput").ap()
    clsTb_d = nc.dram_tensor("clsTb", [128, ET, 16], BF, kind="ExternalInput").ap()
    mw8_d = nc.dram_tensor("mw8", [128, ET, G], F8, kind="ExternalInput").ap()
    cw8_d = nc.dram_tensor("cw8", [1, G], F8, kind="ExternalInput").ap()
    idext8_d = nc.dram_tensor("idext8", [128, 2, 16], F8, kind="ExternalInput").ap()
    extn8_d = nc.dram_tensor("extn8", [BL, D], F8, kind="ExternalInput").ap()
    wh8_d = nc.dram_tensor("wh8", [128, ET, G], F8, kind="ExternalInput").ap()
    h0t8_d = nc.dram_tensor("h0t8", [128, ET, 16], F8, kind="ExternalInput").ap()
    h0b_d = nc.dram_tensor("h0b", [BL, D], BF, kind="ExternalInput").ap()
    mg_d = nc.dram_tensor("mg", [128, ET, D], BF, kind="ExternalInput").ap()
    cgcol_d = nc.dram_tensor("cgcol", [128, ET], FP, kind="ExternalInput").ap()
    w0_d = nc.dram_tensor("w0", [128, ET, D], BF, kind="ExternalInput").ap()
    b0col_d = nc.dram_tensor("b0col", [128, ET], FP, kind="ExternalInput").ap()
    w1t_d = nc.dram_tensor("w1t", [128, ET, D], BF, kind="ExternalInput").ap()
    pmask_d = nc.dram_tensor("pmaskb", [64, 2], BF, kind="ExternalInput").ap()
    xn_d = nc.dram_tensor("xn", [BL, NK, D], BF, kind="ExternalInput").ap()
    xt8_d = nc.dram_tensor("xt8", [BL, D, 912], F8, kind="ExternalInput").ap()
    m2_d = nc.dram_tensor("m2", [128, 2 * ET, D], BF, kind="ExternalInput").ap()
    vconcol_d = nc.dram_tensor("vconcol", [128, ET], FP, kind="ExternalInput").ap()
    f1_d = nc.dram_tensor("f1", [128, 12, 1024], BF, kind="ExternalInput").ap()
    b1row_d = nc.dram_tensor("b1row", [1, 1024], BF, kind="ExternalInput").ap()
    f2_d = nc.dram_tensor("f2", [128, 8, 512], BF, kind="ExternalInput").ap()
    b2row_d = nc.dram_tensor("b2row", [1, 512], BF, kind="ExternalInput").ap()
    f3_d = nc.dram_tensor("f3", [128, 4, 1024], BF, kind="ExternalInput").ap()
    b3row_d = nc.dram_tensor("b3row", [1, 1024], BF, kind="ExternalInput").ap()
    out_d = nc.dram_tensor("out", [BL, 1024], FP, kind="ExternalOutput").ap()

    with tile.TileContext(nc) as tc, ExitStack() as ctx:
        cpool = ctx.enter_context(tc.tile_pool(name="const", bufs=1))
        tail = ctx.enter_context(tc.tile_pool(name="tail", bufs=1))
        psT = ctx.enter_context(tc.tile_pool(name="psT", bufs=1, space="PSUM"))

        identf = cpool.tile([128, 128], FP, tag="identf")
        make_identity(nc, identf[:])
        identb = cpool.tile([128, 128], BF, tag="identb")
        nc.vector.tensor_copy(identb[:], identf[:])
        ones8 = cpool.tile([1, 16], F8, tag="ones8")
        nc.vector.memset(ones8[:], 1.0)
        onesb = cpool.tile([1, 16], BF, tag="onesb")
        nc.vector.memset(onesb[:], 1.0)
        clsTb = cpool.tile([128, ET, 16], BF, tag="clsTb")
        nc.sync.dma_start(clsTb[:], clsTb_d[:])
        qembT = cpool.tile([128, ET, BL, T], BF, tag="qembT")
        goutT = cpool.tile([128, ET, BL], BF, tag="goutT")
        QtT8 = cpool.tile([128, ET, 512], F8, tag="QtT8")
        pcxT = cpool.tile([128, ET, 2 * BL], BF, tag="pcxT")
        wxn_sb = cpool.tile([BL, D], BF, tag="wxn_sb")
        pmaskb = cpool.tile([64, 2], BF, tag="pmaskb")
        nc.sync.dma_start(pmaskb[:], pmask_d[:])

        with tc.tile_pool(name="phW", bufs=1) as phW, \
             tc.tile_pool(name="pgru", bufs=1) as pgru, \
             tc.tile_pool(name="g1", bufs=2) as g1, \
             tc.tile_pool(name="phA", bufs=1) as phA:
            # --- critical-path DMAs first: phase A operands
            clsT8 = phA.tile([128, ET, 16], F8, tag="clsT8")
            nc.sync.dma_start(clsT8[:], clsT8_d[:])
            mw8 = phA.tile([128, ET, G], F8, tag="mw8")
            for tt in range(ET):
                nc.sync.dma_start(mw8[:, tt, :], mw8_d[:, tt, :])
            cw8 = phA.tile([1, G], F8, tag="cw8")
            nc.sync.dma_start(cw8[:], cw8_d[:])
            # --- GRU weights (needed ~10us in)
            wh8 = pgru.tile([128, ET, G], F8, tag="wh8")
            for tt in range(ET):
                nc.sync.dma_start(wh8[:, tt, :], wh8_d[:, tt, :])
            idext8 = pgru.tile([128, 2, 16], F8, tag="idext8")
            nc.sync.dma_start(idext8[:], idext8_d[:])
            ext8 = pgru.tile([128, 2, G], F8, tag="ext8")
            nc.vector.memset(ext8[:].rearrange("p a b -> p (a b)"), 0.0)
            nc.sync.dma_start(ext8[0:BL, 0, 2 * D:3 * D], extn8_d[:])
            # --- mid-phase weights (gout / phase C), stream during GRU
            mg = phW.tile([128, ET, D], BF, tag="mg")
            for tt in range(ET):
                nc.sync.dma_start(mg[:, tt, :], mg_d[:, tt, :])
            cgcol = phW.tile([128, ET], FP, tag="cgcol")
            nc.sync.dma_start(cgcol[:], cgcol_d[:])
            w0 = phW.tile([128, ET, D], BF, tag="w0")
            for tt in range(ET):
                nc.sync.dma_start(w0[:, tt, :], w0_d[:, tt, :])
            b0col = phW.tile([128, ET], FP, tag="b0col")
            nc.sync.dma_start(b0col[:], b0col_d[:])
            w1tt = phW.tile([128, ET, D], BF, tag="w1t")
            for tt in range(ET):
                nc.sync.dma_start(w1tt[:, tt, :], w1t_d[:, tt, :])
            # --- late-phase weights (phase E), stream during GRU
            m2 = tail.tile([128, 2 * ET, D], BF, tag="m2")
            for tt in range(2 * ET):
                nc.sync.dma_start(m2[:, tt, :], m2_d[:, tt, :])
            vconcol = tail.tile([128, ET], FP, tag="vconcol")
            nc.sync.dma_start(vconcol[:], vconcol_d[:])
            f1 = tail.tile([128, 12, 1024], BF, tag="f1")
            for tt in range(12):
                nc.sync.dma_start(f1[:, tt, :], f1_d[:, tt, :])
            f2 = tail.tile([128, 8, 512], BF, tag="f2")
            for tt in range(8):
                nc.sync.dma_start(f2[:, tt, :], f2_d[:, tt, :])
            f3 = tail.tile([128, 4, 1024], BF, tag="f3")
            for tt in range(4):
                nc.sync.dma_start(f3[:, tt, :], f3_d[:, tt, :])
            b1row = tail.tile([1, 1024], BF, tag="b1row")
            nc.sync.dma_start(b1row[:], b1row_d[:])
            b2row = tail.tile([1, 512], BF, tag="b2row")
            nc.sync.dma_start(b2row[:], b2row_d[:])
            b3row = tail.tile([1, 1024], BF, tag="b3row")
            nc.sync.dma_start(b3row[:], b3row_d[:])

            with tc.tile_pool(name="psG", bufs=1, space="PSUM") as psG:
                # ============ phase A: wx = cls@MW + cw ====================

                def gate_mms(lhsT, rhs_w, prz, pn):
                    """emit the 20 DR matmuls for one full [8, 2304] gate set"""
                    subs = [(prz, 0, 512, 0), (prz, 512, 512, 512),
                            (prz, 1024, 512, 1024), (pn, 0, 512, 1536),
                            (pn, 512, 256, 2048)]
                    out = []
                    for (dst, o0, w_, j0) in subs:
                        for g in range(3):
                            nc.tensor.matmul(dst[:, o0:o0 + w_],
                                             lhsT[:, 2 * g:2 * g + 2, :],
                                             rhs_w[:, 2 * g:2 * g + 2, j0:j0 + w_],
                                             start=(g == 0), stop=False,
                                             perf_mode=DR)
                        out.append((dst, o0, w_, j0))
                    return out

                przA = psG.tile([16, 1536], FP, tag="prz")
                pnA = psG.tile([16, 768], FP, tag="pn")
                for (dst, o0, w_, j0) in gate_mms(clsT8, mw8, przA, pnA):
                    nc.tensor.matmul(dst[:, o0:o0 + w_], ones8[:1, :],
                                     cw8[:, j0:j0 + w_], start=False, stop=True)
                nc.scalar.activation(ext8[0:BL, 0, 0:1536], przA[:BL, :],
                                     AF.Copy)
                nc.scalar.activation(wxn_sb[:], pnA[:BL, :], AF.Copy)

                # ============ phase B: GRU =================================
                hT8s = [pgru.tile([128, ET, 16], F8, tag=f"hT8{i}",
                                  name=f"hT8{i}") for i in range(2)]
                nc.sync.dma_start(hT8s[0][:], h0t8_d[:])
                nc.vector.memset(hT8s[1][:, :, BL:16], 0.0)
                hnat = g1.tile([BL, D], BF, tag="hnat")
                nc.sync.dma_start(hnat[:], h0b_d[:])

                for t in range(KSTEPS):
                    hT8 = hT8s[t % 2]
                    hT8n = hT8s[(t + 1) % 2]
                    prz = psG.tile([16, 1536], FP, tag="prz")
                    pn = psG.tile([16, 768], FP, tag="pn")
                    subs = [(prz, 0, 512, 0), (prz, 512, 512, 512),
                            (prz, 1024, 512, 1024), (pn, 0, 512, 1536),
                            (pn, 512, 256, 2048)]
                    rz = g1.tile([BL, 2 * D], BF, tag="rz")
                    u_ = g1.tile([BL, D], BF, tag="u")
                    w_ = g1.tile([BL, D], BF, tag="w")
                    t1 = g1.tile([BL, D], BF, tag="t1")
                    t2 = g1.tile([BL, D], BF, tag="t2")
                    nt_ = g1.tile([BL, D], BF, tag="nt")
                    v_ = g1.tile([BL, D], BF, tag="v")
                    hnew = g1.tile([BL, D], BF, tag="hnat")

                    def sub_mms(si):
                        dst, o0, w__, j0 = subs[si]
                        for g in range(3):
                            nc.tensor.matmul(dst[:, o0:o0 + w__],
                                             hT8[:, 2 * g:2 * g + 2, :],
                                             wh8[:, 2 * g:2 * g + 2, j0:j0 + w__],
                                             start=(g == 0), stop=False,
                                             perf_mode=DR)
                        nc.tensor.matmul(dst[:, o0:o0 + w__], idext8[:],
                                         ext8[:, :, j0:j0 + w__],
                                         start=False, stop=True, perf_mode=DR)

                    sub_mms(0)
                    sub_mms(1)
                    # r ready -> sigmoid overlaps remaining stream
                    nc.scalar.activation(rz[:, 0:D], prz[:BL, 0:D], AF.Sigmoid,
                                         scale=1.0 / SG)
                    sub_mms(2)
                    nc.scalar.activation(rz[:, D:2 * D], prz[:BL, D:2 * D],
                                         AF.Sigmoid, scale=1.0 / SG)
                    # z-dependent terms during the n-chunk stream
                    nc.vector.tensor_mul(u_[:], rz[:, D:2 * D], hnat[:])
                    nc.vector.tensor_scalar(w_[:], rz[:, D:2 * D], -1.0, 1.0,
                                            OP.mult, OP.add)
                    sub_mms(3)
                    nc.vector.tensor_mul(t1[:, 0:512], rz[:, 0:512],
                                         pn[:BL, 0:512])
                    nc.vector.tensor_add(t2[:, 0:512], t1[:, 0:512],
                                         wxn_sb[:, 0:512])
                    nc.scalar.activation(nt_[:, 0:512], t2[:, 0:512], AF.Tanh,
                                         scale=1.0 / SG)
                    sub_mms(4)
                    for dd in range(NDUM):
                        pdum = psT.tile([16, 512], FP, tag="pt",
                                        padded_shape=[128, 1024])
                        nc.tensor.matmul(pdum[:], idext8[:],
                                         ext8[:, :, 0:512], start=True,
                                         stop=True, perf_mode=DR)
                    nc.vector.tensor_mul(t1[:, 512:768], rz[:, 512:768],
                                         pn[:BL, 512:768])
                    nc.vector.tensor_add(t2[:, 512:768], t1[:, 512:768],
                                         wxn_sb[:, 512:768])
                    nc.scalar.activation(nt_[:, 512:768], t2[:, 512:768],
                                         AF.Tanh, scale=1.0 / SG)
                    nc.vector.tensor_mul(v_[:], w_[:], nt_[:])
                    nc.vector.tensor_add(hnew[:], v_[:], u_[:])
                    pt = psT.tile([128, 64], BF, tag="pt",
                                  padded_shape=[128, 1024])
                    for kt in range(ET):
                        nc.tensor.matmul(pt[:, 8 * kt:8 * kt + 8],
                                         hnew[:, 128 * kt:128 * (kt + 1)],
                                         identb[:BL, :BL], is_transpose=True,
                                         skip_group_check=True)
                    nc.vector.tensor_copy(
                        hT8n[:, :, 0:BL],
                        pt[:, :48].rearrange("p (a b) -> p a b", b=BL))
                    nc.scalar.copy(
                        qembT[:, :, :, t].rearrange("p a b -> p (a b)"),
                        pt[:, :48])
                    hnat = hnew

            # ============ gout = cls@Mg + cg (feature-major) ==============
            with tc.tile_pool(name="psC", bufs=2, space="PSUM") as psC:
                for mt in range(ET):
                    p = psC.tile([128, BL], FP, tag="pg",
                                 padded_shape=[128, 512])
                    for kt in range(ET):
                        nc.tensor.matmul(p[:],
                                         mg[:, kt, 128 * mt:128 * (mt + 1)],
                                         clsTb[:, kt, 0:BL],
                                         start=(kt == 0), stop=(kt == ET - 1))
                    nc.vector.tensor_scalar(goutT[:, mt, :], p[:],
                                            cgcol[:, mt:mt + 1], None, OP.add)

                # ============ phase C: QT, QtT8 ===========================
                if PHASES >= 2:
                    qflat = qembT[:].rearrange("p a b t -> p a (b t)")
                    QT = g1.tile([128, ET, BL * T], BF, tag="QT")
                    for mt in range(ET):
                        p = psC.tile([128, BL * T], FP, tag="pc",
                                     padded_shape=[128, 512])
                        for kt in range(ET):
                            nc.tensor.matmul(
                                p[:], w0[:, kt, 128 * mt:128 * (mt + 1)],
                                qflat[:, kt, :],
                                start=(kt == 0), stop=(kt == ET - 1))
                        nc.vector.tensor_scalar(QT[:, mt, :], p[:],
                                                b0col[:, mt:mt + 1], None,
                                                OP.add)
                    for hd in range(NH):
                        for mt in range(ET):
                            p = psC.tile([128, BL * T], FP, tag="pc",
                                         padded_shape=[128, 512])
                            for i in range(3):
                                kt = 3 * hd + i
                                nc.tensor.matmul(
                                    p[:], w1tt[:, kt, 128 * mt:128 * (mt + 1)],
                                    QT[:, kt, :],
                                    start=(i == 0), stop=(i == 2))
                            dst = QtT8[:, mt, :].rearrange(
                                "p (b h2 t) -> p b h2 t", h2=NH, t=T)[:, :, hd, :]
                            src = p[:].rearrange("p (b t) -> p b t", t=T)
                            nc.scalar.activation(dst, src, AF.Copy, scale=SQ)

        # ================= phase D: per-b attention (sw-pipelined) ========
        if PHASES >= 3:
            with tc.tile_pool(name="xb", bufs=3) as xb, \
                 tc.tile_pool(name="ab", bufs=2) as ab, \
                 tc.tile_pool(name="psS", bufs=2, space="PSUM") as psS:
                state = {}

                def emit_scores(b):
                    xn_t = xb.tile([128, 8, D], BF, tag="xn")
                    nc.gpsimd.memset(xn_t[:, 7, :], 0.0)
                    for c, (k0, kw) in enumerate(KC):
                        nc.gpsimd.dma_start(xn_t[:kw, c, :],
                                            xn_d[b, k0:k0 + kw, :])
                    xt_t = xb.tile([128, ET, 912], F8, tag="xt")
                    for tt in range(ET):
                        nc.gpsimd.dma_start(xt_t[:, tt, :],
                                            xt8_d[b, 128 * tt:128 * (tt + 1), :])
                    att = ab.tile([64, NK], BF, tag="att")
                    zacc = ab.tile([64, 2], FP, tag="zacc")
                    for ci, (n0, nw) in enumerate(CH_NK):
                        p = psS.tile([64, 512], FP, tag="s")
                        for g in range(3):
                            nc.tensor.matmul(
                                p[:, :nw],
                                QtT8[:, 2 * g:2 * g + 2, 64 * b:64 * b + 64],
                                xt_t[:, 2 * g:2 * g + 2, n0:n0 + nw],
                                start=(g == 0), stop=(g == 2), perf_mode=DR)
                        nc.scalar.activation(att[:, n0:n0 + nw], p[:, :nw],
                                             AF.Exp, scale=ESC,
                                             accum_out=zacc[:, ci:ci + 1])
                    state[b] = (xn_t, att, zacc)

                def emit_tail(b):
                    xn_t, att, zacc = state.pop(b)
                    zs = ab.tile([64, 1], FP, tag="zs")
                    nc.vector.tensor_add(zs[:], zacc[:, 0:1], zacc[:, 1:2])
                    rcp = ab.tile([64, 1], FP, tag="rcp")
                    nc.vector.reciprocal(rcp[:], zs[:])
                    wm = ab.tile([64, 2], BF, tag="wm")
                    nc.vector.tensor_scalar(wm[:], pmaskb[:], rcp[:, 0:1],
                                            None, OP.mult)
                    pa_sb = ab.tile([2, NK], BF, tag="pa_sb")
                    for ci, (n0, nw) in enumerate(CH_NK):
                        p2 = psS.tile([2, 512], FP, tag="p2")
                        nc.tensor.matmul(p2[:, :nw], wm[:], att[:, n0:n0 + nw],
                                         start=True, stop=True)
                        nc.vector.tensor_copy(pa_sb[:, n0:n0 + nw],
                                              p2[:, :nw])
                    paT = ab.tile([128, len(KC), 2], BF, tag="paT")
                    nc.gpsimd.memset(paT[:].rearrange("p a b -> p (a b)"), 0.0)
                    ptp = psT.tile([128, 16], BF, tag="pt",
                                   padded_shape=[128, 1024])
                    for c, (k0, kw) in enumerate(KC):
                        nc.tensor.matmul(ptp[:kw, 2 * c:2 * c + 2],
                                         pa_sb[:, k0:k0 + kw], identb[:2, :2],
                                         is_transpose=True,
                                         skip_group_check=True)
                        nc.vector.tensor_copy(paT[:kw, c, :],
                                              ptp[:kw, 2 * c:2 * c + 2])
                    pcx = ab.tile([2, D], BF, tag="pcx")
                    for ci, (n0, nw) in enumerate(CH_D):
                        p = psS.tile([2, 512], FP, tag="v")
                        for c in range(len(KC)):
                            nc.tensor.matmul(p[:, :nw], paT[:, c, :],
                                             xn_t[:, c, n0:n0 + nw],
                                             start=(c == 0),
                                             stop=(c == len(KC) - 1))
                        nc.vector.tensor_copy(pcx[:, n0:n0 + nw], p[:, :nw])
                    ptc = psT.tile([128, 16], BF, tag="pt",
                                   padded_shape=[128, 1024])
                    for kt in range(ET):
                        nc.tensor.matmul(ptc[:, 2 * kt:2 * kt + 2],
                                         pcx[:, 128 * kt:128 * (kt + 1)],
                                         identb[:2, :2], is_transpose=True,
                                         skip_group_check=True)
                    nc.vector.tensor_copy(
                        pcxT[:, :, 2 * b:2 * b + 2],
                        ptc[:, :2 * ET].rearrange("p (a c) -> p a c", c=2))

                for b in range(BL):
                    emit_scores(b)
                    if b > 0:
                        emit_tail(b - 1)
                emit_tail(BL - 1)

        # ================= phase E: projections + MLP =====================
        if PHASES >= 4:
            with tc.tile_pool(name="psE", bufs=2, space="PSUM") as psE:
                loT = tail.tile([128, ET, BL], BF, tag="loT")
                for mt in range(ET):
                    p = psE.tile([128, BL], FP, tag="pe",
                                 padded_shape=[128, 512])
                    k = 0
                    for hd in range(NH):
                        for kt in range(ET):
                            pcv = pcxT[:].rearrange(
                                "p a (b h) -> p a b h", h=2)[:, kt, :, hd]
                            nc.tensor.matmul(
                                p[:], m2[:, ET * hd + kt, 128 * mt:128 * (mt + 1)],
                                pcv, start=(k == 0), stop=(k == 2 * ET - 1))
                            k += 1
                    nc.vector.tensor_scalar(loT[:, mt, :], p[:],
                                            vconcol[:, mt:mt + 1], None,
                                            OP.add)

                y1b = tail.tile([BL, 1024], BF, tag="y1b")
                for ch in range(2):
                    p = psE.tile([BL, 512], FP, tag="pe")
                    for kt in range(12):
                        lhs = loT[:, kt, :] if kt < ET else goutT[:, kt - ET, :]
                        nc.tensor.matmul(p[:], lhs,
                                         f1[:, kt, 512 * ch:512 * (ch + 1)],
                                         start=(kt == 0), stop=False)
                    nc.tensor.matmul(p[:], onesb[:1, :BL],
                                     b1row[:, 512 * ch:512 * (ch + 1)],
                                     start=False, stop=True)
                    nc.scalar.activation(y1b[:, 512 * ch:512 * (ch + 1)], p[:],
                                         AF.Copy)
                pt1 = psT.tile([128, 64], BF, tag="pt",
                               padded_shape=[128, 1024])
                for kt in range(8):
                    nc.tensor.matmul(pt1[:, 8 * kt:8 * kt + 8],
                                     y1b[:, 128 * kt:128 * (kt + 1)],
                                     identb[:BL, :BL], is_transpose=True,
                                     skip_group_check=True)
                y1T = tail.tile([128, 8, BL], BF, tag="y1T")
                nc.vector.tensor_copy(y1T[:].rearrange("p a b -> p (a b)"),
                                      pt1[:, :64])

                p = psE.tile([BL, 512], FP, tag="pe")
                for kt in range(8):
                    nc.tensor.matmul(p[:], y1T[:, kt, :], f2[:, kt, :],
                                     start=(kt == 0), stop=False)
                nc.tensor.matmul(p[:], onesb[:1, :BL], b2row[:],
                                 start=False, stop=True)
                y2b = tail.tile([BL, 512], BF, tag="y2b")
                nc.scalar.activation(y2b[:], p[:], AF.Relu)
                pt2 = psT.tile([128, 32], BF, tag="pt",
                               padded_shape=[128, 1024])
                for kt in range(4):
                    nc.tensor.matmul(pt2[:, 8 * kt:8 * kt + 8],
                                     y2b[:, 128 * kt:128 * (kt + 1)],
                                     identb[:BL, :BL], is_transpose=True,
                                     skip_group_check=True)
                y2T = tail.tile([128, 4, BL], BF, tag="y2T")
                nc.vector.tensor_copy(y2T[:].rearrange("p a b -> p (a b)"),
                                      pt2[:, :32])

                ynat = tail.tile([BL, 1024], FP, tag="ynat")
                for ch in range(2):
                    p = psE.tile([BL, 512], FP, tag="pe")
                    for kt in range(4):
                        nc.tensor.matmul(p[:], y2T[:, kt, :],
                                         f3[:, kt, 512 * ch:512 * (ch + 1)],
                                         start=(kt == 0), stop=False)
                    nc.tensor.matmul(p[:], onesb[:1, :BL],
                                     b3row[:, 512 * ch:512 * (ch + 1)],
                                     start=False, stop=True)
                    nc.vector.tensor_copy(ynat[:, 512 * ch:512 * (ch + 1)],
                                          p[:])
                nc.sync.dma_start(out_d[:, :], ynat[:])

    nc.compile()
    return nc


_NC = None


def _bf(x):
    return np.ascontiguousarray(x).astype(ml_dtypes.bfloat16)


def _f8(x):
    return np.ascontiguousarray(x).astype(ml_dtypes.float8_e4m3)


def _tile6(w):
    """[768, J] -> [128, 6, J] with [p, t, j] = w[128t+p, j]"""
    J = w.shape[1]
    return np.ascontiguousarray(w.reshape(ET, 128, J).transpose(1, 0, 2))


def make_in_maps(inputs):
    f32 = np.float32
    img = np.asarray(inputs["image_local_embeds"], f32)
    h0 = np.asarray(inputs["h0"], f32)
    w_ih = np.asarray(inputs["gru_w_ih"], f32)
    w_hh = np.asarray(inputs["gru_w_hh"], f32)
    b_ih = np.asarray(inputs["gru_b_ih"], f32)
    b_hh = np.asarray(inputs["gru_b_hh"], f32)
    ga_w = np.asarray(inputs["ga_w"], f32)
    ga_b = np.asarray(inputs["ga_b"], f32)
    ga_pool = np.asarray(inputs["ga_pool"], f32)
    la_w = np.asarray(inputs["la_w"], f32)
    la_b = np.asarray(inputs["la_b"], f32)
    la_pool = np.asarray(inputs["la_pool"], f32)
    go_w = np.asarray(inputs["go_w"], f32)
    go_b = np.asarray(inputs["go_b"], f32)
    go_pool = np.asarray(inputs["go_pool"], f32)
    f1_w = np.asarray(inputs["f1_w"], f32)
    f1_b = np.asarray(inputs["f1_b"], f32)
    f2_w = np.asarray(inputs["f2_w"], f32)
    f2_b = np.asarray(inputs["f2_b"], f32)
    f3_w = np.asarray(inputs["f3_w"], f32)
    f3_b = np.asarray(inputs["f3_b"], f32)

    Mga = ga_pool[0] * (ga_w[2] @ ga_w[3])
    cga = ga_pool[0] * (ga_b[2] @ ga_w[3] + ga_b[3])
    MW = Mga @ w_ih.T
    cw = cga @ w_ih.T + b_ih
    cw[:2 * D] += b_hh[:2 * D]
    Sgo = go_pool.sum()
    Mg = Sgo * (go_w[2] @ go_w[3])
    cg = Sgo * (go_b[2] @ go_w[3] + go_b[3])
    Sla = la_pool.sum()
    M2 = np.stack([la_w[2][:, hd * DK:(hd + 1) * DK]
                   @ la_w[3][hd * DK:(hd + 1) * DK, :] for hd in range(NH)])
    vcon = Sla * (la_b[2] @ la_w[3] + la_b[3])
    W1T = np.ascontiguousarray(la_w[1].T)

    idext = np.zeros((128, 2, 16), f32)
    for b in range(BL):
        idext[b, 0, b] = 1.0
    pmask = np.zeros((64, 2), f32)
    pmask[0:T, 0] = la_pool
    pmask[T:2 * T, 1] = la_pool

    mw8 = _f8(_tile6(SG * MW))
    cw8 = _f8((SG * cw)[None, :])
    wh8 = _f8(_tile6(SG * w_hh.T))
    extn8 = _f8(np.broadcast_to(SG * b_hh[2 * D:], (BL, D)).copy())
    idext8 = _f8(idext)
    mg = _bf(_tile6(Mg))
    cgcol = np.ascontiguousarray(cg.reshape(ET, 128).T)
    w0 = _bf(_tile6(la_w[0]))
    b0col = np.ascontiguousarray(la_b[0].reshape(ET, 128).T)
    w1t = _bf(_tile6(W1T))
    m2 = _bf(np.concatenate([_tile6(M2[0]), _tile6(M2[1])], axis=1))
    vconcol = np.ascontiguousarray(vcon.reshape(ET, 128).T)
    f1p = _bf(f1_w.reshape(12, 128, 1024).transpose(1, 0, 2))
    f2p = _bf(f2_w.reshape(8, 128, 512).transpose(1, 0, 2))
    f3p = _bf(f3_w.reshape(4, 128, 1024).transpose(1, 0, 2))

    in_maps = []
    B = img.shape[0]
    per = B // NCORES
    for c in range(NCORES):
        sl = slice(c * per, (c + 1) * per)
        cls = img[sl, 0, :]
        X = img[sl, 1:, :]
        clsT = np.zeros((128, ET, 16), f32)
        clsT[:, :, :BL] = cls.T.reshape(ET, 128, BL).transpose(1, 0, 2)
        h0c = h0[sl]
        h0t = np.zeros((128, ET, 16), f32)
        h0t[:, :, :BL] = h0c.T.reshape(ET, 128, BL).transpose(1, 0, 2)
        xt = np.zeros((per, D, 912), f32)
        xt[:, :, :NK] = SX * X.transpose(0, 2, 1)
        m = {
            "clsT8": _f8(clsT),
            "clsTb": _bf(clsT),
            "mw8": mw8, "cw8": cw8, "idext8": idext8, "extn8": extn8,
            "wh8": wh8,
            "h0t8": _f8(h0t), "h0b": _bf(h0c),
            "mg": mg, "cgcol": cgcol.astype(f32),
            "w0": w0, "b0col": b0col.astype(f32),
            "w1t": w1t, "pmaskb": _bf(pmask),
            "xn": _bf(X), "xt8": _f8(xt),
            "m2": m2, "vconcol": vconcol.astype(f32),
            "f1": f1p, "b1row": _bf(f1_b[None, :]),
            "f2": f2p, "b2row": _bf(f2_b[None, :]),
            "f3": f3p, "b3row": _bf(f3_b[None, :]),
        }
        in_maps.append(m)
    return in_maps


def kernel(**inputs):
    global _NC
    if _NC is None:
        _NC = build()
    in_maps = make_in_maps(inputs)
    res = run_bass_kernel_spmd(_NC, in_maps, core_ids=list(range(NCORES)))
    return np.concatenate([res.results[c]["out"] for c in range(NCORES)],
                          axis=0)
